# revision 1
# baseline (speedup 1.0000x reference)
r"""Trainium2 Bass kernel for the triangular-DP "MAA layer" problem.

Reference computes, per frame t (T=1024, D=256, L=T+1 counts):
    q_t = (1-p_t) q_{t-1} + p_t shift(q_{t-1})          (Poisson-binomial DP)
    m_t = p_t a m_sh + (1-p_t) m + p_t b q_sh x_t       ([L, D] state)
    out = sum_i m_T[i, :]                               ([D])

Algebraic restructuring used here: with s_t[i] = i*m_t[i], every step is a
polynomial in the (commuting, nilpotent) count-shift operator S, so the whole
scan collapses to

    out[d] = sum_t c_t x[t, d],
    c_t    = p_t * I_t,   I_t = int_0^1 prod_{s != t} ((1-p_s) + p_s u) du.

The integrand is a degree-(T-1) polynomial; K-node Gauss-Legendre quadrature
is exact for K >= T/2 and already converged to the f32 noise floor at K=64
(verified < 3e-6 rel err vs the reference). With f[t,k] = (1-p_t) + p_t u_k:

    G_k = prod_t f[t,k]  (as exp(sum_t ln f))
    c_t = p_t * sum_k (w_k G_k) / f[t,k]
    out = c^T @ x

Device mapping (t on partitions, 8 chunks of 128; k on free dim, K=64):
  - lf[c] = Ln(um1 * p_c + 1)      one fused ScalarE activation per chunk
  - slog  = ones^T @ lf_supertile  one PE matmul; cross-chunk sum on DVE
  - G     = Exp(slog), gw = w*G, broadcast via 1-partition PE matmul
  - rf[c] = Exp(-lf[c])            reciprocal on ScalarE (table reuse)
  - cfin[c] = sum_k (rf*p_c)*gwbc  one DVE scalar_tensor_tensor w/ accum
  - z     = sum_c cfin_c * x_c     DVE chain; out = ones^T @ z (one matmul)

The 8 cores each run the identical full problem (1 MB x DMA each);
replication beats sharding because the 8-core collective latency floor
(~5-10 us) exceeds the whole compute phase.
"""

import numpy as np

T, D, NCH, P, K = 1024, 256, 8, 128, 64
N_CORES = 8

_CACHE = {}


def _gl_nodes_weights():
    nodes, weights = np.polynomial.legendre.leggauss(K)
    u = (nodes + 1.0) * 0.5
    w = weights * 0.5
    return u, w


def _build_program():
    import concourse.bass as bass
    import concourse.bacc as bacc
    import concourse.mybir as mybir
    import concourse.tile as tile

    f32 = mybir.dt.float32
    A = mybir.AluOpType
    ACT = mybir.ActivationFunctionType

    nc = bacc.Bacc("TRN2", target_bir_lowering=False, debug=False,
                   num_devices=N_CORES)

    # aux columns: [um1 (K) | pcol (NCH) | onescol (1)] -> one DMA, one wait
    AUXW = K + NCH + 1
    xall_d = nc.dram_tensor("xall", [P, NCH * D], f32, kind="ExternalInput")
    aux_d = nc.dram_tensor("aux", [P, AUXW], f32, kind="ExternalInput")
    rows_d = nc.dram_tensor("rows", [1, K + P], f32, kind="ExternalInput")
    out_d = nc.dram_tensor("out", [1, D], f32, kind="ExternalOutput")

    with tile.TileContext(nc) as tc:
        with (
            tc.tile_pool(name="sb", bufs=1) as sb,
            tc.tile_pool(name="ps", bufs=1, space=bass.MemorySpace.PSUM) as ps,
        ):
            xall = sb.tile([P, NCH * D], f32, tag="xall")
            aux = sb.tile([P, AUXW], f32, tag="aux")
            rows = sb.tile([1, K + P], f32, tag="rows")
            nc.sync.dma_start(aux[:], aux_d[:])
            nc.sync.dma_start(rows[:], rows_d[:])
            nc.sync.dma_start(xall[:], xall_d[:])
            um1 = aux[:, 0:K]
            pcol = aux[:, K:K + NCH]
            onescol = aux[:, K + NCH:K + NCH + 1]

            # Dummy partition_broadcast: absorbs GpSimd's one-time ucode
            # setup + drain off the critical path
            dumb = sb.tile([8, 1], f32, tag="dumb")
            nc.gpsimd.memset(dumb[0:1, :], 0.0)
            nc.gpsimd.partition_broadcast(dumb[:], dumb[0:1, :], channels=8)

            # Stage A, all chunks batched via stride-0 broadcast APs:
            #   fm1[t,(c,k)] = p[c,t] * um1[k]  (one DVE op)
            #   lf = Ln(fm1 + 1)                (one ScalarE op, bias fused)
            um1_rep = um1.unsqueeze(1).broadcast_to([P, NCH, K])
            p_rep = pcol.unsqueeze(2).broadcast_to([P, NCH, K])
            fm1 = sb.tile([P, NCH * K], f32, tag="fm1")
            nc.vector.tensor_tensor(fm1.rearrange("p (c k) -> p c k", c=NCH),
                                    um1_rep, p_rep, op=A.mult)
            lfbig = sb.tile([P, NCH * K], f32, tag="lfbig")
            nc.scalar.activation(lfbig[:], fm1[:], ACT.Ln, bias=1.0)

            # slog_k = sum_t ln f: cross-chunk sum on DVE (strided view),
            # then one 64-col PE matmul over partitions
            lfsum = sb.tile([P, K], f32, tag="lfsum")
            nc.vector.tensor_reduce(
                lfsum[:], lfbig.rearrange("p (c k) -> p k c", c=NCH),
                axis=mybir.AxisListType.X, op=A.add)
            slog_ps = ps.tile([1, K], f32, tag="slog")
            nc.tensor.matmul(slog_ps[:], onescol[:], lfsum[:],
                             start=True, stop=True)

            # G = exp(slog); gw = w * G; partition-broadcast on GpSimd
            g = sb.tile([1, K], f32, tag="g")
            nc.scalar.activation(g[:], slog_ps[:], ACT.Exp)
            gw = sb.tile([1, K], f32, tag="gw")
            nc.vector.tensor_tensor(gw[:], g[:], rows[:, 0:K], op=A.mult)
            gwbc = sb.tile([P, K], f32, tag="gwbc")
            nc.gpsimd.partition_broadcast(gwbc[:], gw[:], channels=P)

            # rf = exp(-lf) = 1/f, one ScalarE op over all chunks (emitted
            # after G so the small critical-path Exp runs first)
            rfbig = sb.tile([P, NCH * K], f32, tag="rfbig")
            nc.scalar.activation(rfbig[:], lfbig[:], ACT.Exp, scale=-1.0)

            # cfin[t,c] = p[c,t] * sum_k rf * gwbc  (q product + k-reduce)
            gwbc_rep = gwbc.unsqueeze(1).broadcast_to([P, NCH, K])
            q = sb.tile([P, NCH * K], f32, tag="q")
            nc.vector.tensor_tensor(q.rearrange("p (c k) -> p c k", c=NCH),
                                    rfbig.rearrange("p (c k) -> p c k", c=NCH),
                                    gwbc_rep, op=A.mult)
            cfin8 = sb.tile([P, NCH], f32, tag="cfin8")
            nc.vector.tensor_reduce(
                cfin8[:], q.rearrange("p (c k) -> p c k", c=NCH),
                axis=mybir.AxisListType.X, op=A.add)
            cfin = sb.tile([P, NCH], f32, tag="cfin")
            nc.vector.tensor_tensor(cfin[:], cfin8[:], pcol[:], op=A.mult)

            # z[t,d] = sum_c cfin[t,c] * x[t,c,d]: one product with a
            # transposed (d,c) write, then a contiguous innermost-c reduce
            zz = sb.tile([P, NCH * D], f32, tag="zz")
            zz_t = zz.rearrange("p (d c) -> p c d", c=NCH)
            cfin_rep = cfin.unsqueeze(2).broadcast_to([P, NCH, D])
            nc.vector.tensor_tensor(zz_t,
                                    xall.rearrange("p (c d) -> p c d", c=NCH),
                                    cfin_rep, op=A.mult)
            z = sb.tile([P, D], f32, tag="z")
            nc.vector.tensor_reduce(
                z[:], zz.rearrange("p (d c) -> p d c", c=NCH),
                axis=mybir.AxisListType.X, op=A.add)

            out_ps = ps.tile([1, D], f32, tag="out")
            nc.tensor.matmul(out_ps[:], onescol[:], z[:],
                             start=True, stop=True)
            out_sb = sb.tile([1, D], f32, tag="outsb")
            nc.vector.tensor_copy(out_sb[:], out_ps[:])
            nc.sync.dma_start(out_d[:], out_sb[:])

    nc.compile()
    return nc


def _make_in_map(p, x):
    p = np.ascontiguousarray(np.asarray(p, dtype=np.float32)).reshape(T)
    x = np.ascontiguousarray(np.asarray(x, dtype=np.float32)).reshape(T, D)
    u, w = _gl_nodes_weights()
    um1bc = np.tile((u - 1.0).astype(np.float32)[None, :], (P, 1))
    pcol = np.ascontiguousarray(p.reshape(NCH, P).T)
    onescol = np.ones((P, 1), np.float32)
    aux = np.ascontiguousarray(
        np.concatenate([um1bc, pcol, onescol], axis=1))
    rows = np.concatenate([w.astype(np.float32),
                           np.ones(P, np.float32)]).reshape(1, K + P)
    xall = np.ascontiguousarray(
        x.reshape(NCH, P, D).transpose(1, 0, 2).reshape(P, NCH * D))
    return {
        "xall": xall,
        "aux": aux,
        "rows": rows,
    }


def _run(p, x, trace=False, tmpdir=None):
    from concourse.bass_utils import run_bass_kernel_spmd

    if "nc" not in _CACHE:
        _CACHE["nc"] = _build_program()
    nc = _CACHE["nc"]
    in_map = _make_in_map(p, x)
    in_maps = [in_map for _ in range(N_CORES)]
    res = run_bass_kernel_spmd(nc, in_maps, list(range(N_CORES)),
                               trace=trace, tmpdir=tmpdir)
    out = np.asarray(res.results[0]["out"], dtype=np.float32).reshape(D)
    return out, res


def kernel(p, x):
    out, _ = _run(p, x, trace=False)
    return out



# revision 2
# speedup vs baseline: 1.2720x; 1.2720x over previous
r"""Trainium2 Bass kernel for the triangular-DP "MAA layer" problem.

Reference computes, per frame t (T=1024, D=256, L=T+1 counts):
    q_t = (1-p_t) q_{t-1} + p_t shift(q_{t-1})          (Poisson-binomial DP)
    m_t = p_t a m_sh + (1-p_t) m + p_t b q_sh x_t       ([L, D] state)
    out = sum_i m_T[i, :]                               ([D])

Algebraic restructuring: the whole scan collapses to

    out[d] = sum_t c_t x[t, d],
    c_t    = p_t * I_t,   I_t = int_0^1 prod_{s != t} ((1-p_s) + p_s u) du.

K-node Gauss-Legendre quadrature (K=64 >= T/2 not needed; converged at 64)
with f[t,k] = 1 + p_t (u_k - 1):

    slog_k = sum_t ln f[t,k]
    c_t    = p_t * sum_k exp(slog_k + ln w_k - ln f[t,k])
    out    = c^T @ x

Device mapping (t on partitions, 8 chunks of 128; k on free dim, K=64),
replicated on all 8 cores (collective latency floor exceeds compute):
  - p arrives as [8,128] (+identity cols) -> one PE matmul transposes to pcol
  - um1/lnw arrive as one [1,128] row; broadcasts via 1-partition PE matmuls
  - single Ln (bias=1) and single Exp on ScalarE; the Exp table load hides
    behind the DVE reduce + PE slog matmul
  - final contraction: 8 accumulating bf16 PE matmuls (x shipped as bf16)
"""

import numpy as np

T, D, NCH, P, K = 1024, 256, 8, 128, 64
N_CORES = 8

_CACHE = {}


def _gl_nodes_weights():
    nodes, weights = np.polynomial.legendre.leggauss(K)
    u = (nodes + 1.0) * 0.5
    w = weights * 0.5
    return u, w


def _build_program():
    import concourse.bass as bass
    import concourse.bacc as bacc
    import concourse.mybir as mybir
    import concourse.tile as tile

    f32 = mybir.dt.float32
    bf16 = mybir.dt.bfloat16
    A = mybir.AluOpType
    ACT = mybir.ActivationFunctionType

    nc = bacc.Bacc("TRN2", target_bir_lowering=False, debug=False,
                   num_devices=N_CORES)

    aux1_d = nc.dram_tensor("aux1", [1, 2 * K], f32, kind="ExternalInput")
    p8_d = nc.dram_tensor("p8", [NCH, P + NCH], f32, kind="ExternalInput")
    xa_d = nc.dram_tensor("xa", [P, NCH * D], bf16, kind="ExternalInput")
    out_d = nc.dram_tensor("out", [1, D], f32, kind="ExternalOutput")

    with tile.TileContext(nc) as tc:
        with (
            tc.tile_pool(name="sb", bufs=1) as sb,
            tc.tile_pool(name="ps", bufs=1, space=bass.MemorySpace.PSUM) as ps,
        ):
            aux1 = sb.tile([1, 2 * K], f32, tag="aux1")
            p8 = sb.tile([NCH, P + NCH], f32, tag="p8")
            xa = sb.tile([P, NCH * D], bf16, tag="xa")
            nc.sync.dma_start(aux1[:], aux1_d[:])
            nc.sync.dma_start(p8[:], p8_d[:])
            nc.sync.dma_start(xa[:], xa_d[:])
            um1row = aux1[:, 0:K]
            lnwrow = aux1[:, K:2 * K]

            onesrow = sb.tile([1, P], f32, tag="onesrow")
            onescol = sb.tile([P, 1], f32, tag="onescol")
            nc.gpsimd.memset(onesrow[:], 1.0)
            nc.gpsimd.memset(onescol[:], 1.0)

            # pcol[t, c] = p[c*128 + t] via PE transpose (p8 @ identity)
            pcol_ps = ps.tile([P, NCH], f32, tag="pcol_ps")
            nc.tensor.matmul(pcol_ps[:], p8[:, 0:P], p8[:, P:P + NCH],
                             start=True, stop=True)
            pcol = sb.tile([P, NCH], f32, tag="pcol")
            nc.vector.tensor_copy(pcol[:], pcol_ps[:])

            # um1bc[t, k] = u_k - 1 broadcast across partitions (PE outer prod)
            um1bc_ps = ps.tile([P, K], f32, tag="um1bc_ps")
            nc.tensor.matmul(um1bc_ps[:], onesrow[:], um1row,
                             start=True, stop=True)

            # fm1[t,(c,k)] = pcol[t,c] * um1[k]
            fm1 = sb.tile([P, NCH * K], f32, tag="fm1")
            um1_rep = um1bc_ps.unsqueeze(1).broadcast_to([P, NCH, K])
            p_rep = pcol.unsqueeze(2).broadcast_to([P, NCH, K])
            nc.vector.tensor_tensor(fm1.rearrange("p (c k) -> p c k", c=NCH),
                                    um1_rep, p_rep, op=A.mult)

            # lf = Ln(fm1 + 1)
            lfbig = sb.tile([P, NCH * K], f32, tag="lfbig")
            nc.scalar.activation(lfbig[:], fm1[:], ACT.Ln, bias=1.0)

            # lfsum[t, k] = sum_c lf[t,(c,k)] via 3 contiguous halving adds
            h1 = sb.tile([P, 4 * K], f32, tag="h1")
            nc.vector.tensor_tensor(h1[:], lfbig[:, 0:4 * K],
                                    lfbig[:, 4 * K:8 * K], op=A.add)
            h2 = sb.tile([P, 2 * K], f32, tag="h2")
            nc.vector.tensor_tensor(h2[:], h1[:, 0:2 * K], h1[:, 2 * K:4 * K],
                                    op=A.add)
            lfsum = sb.tile([P, K], f32, tag="lfsum")
            nc.vector.tensor_tensor(lfsum[:], h2[:, 0:K], h2[:, K:2 * K],
                                    op=A.add)

            # slog_k = sum_t lfsum[t, k]  (PE partition reduce)
            slog_ps = ps.tile([1, K], f32, tag="slog_ps")
            nc.tensor.matmul(slog_ps[:], onescol[:], lfsum[:],
                             start=True, stop=True)

            # slnw = slog + ln w ; broadcast to partitions on PE
            slnw = sb.tile([1, K], f32, tag="slnw")
            nc.vector.tensor_tensor(slnw[:], slog_ps[:], lnwrow, op=A.add)
            argbc_ps = ps.tile([P, K], f32, tag="argbc_ps")
            nc.tensor.matmul(argbc_ps[:], onesrow[:], slnw[:],
                             start=True, stop=True)

            # arg = (slog + lnw) - lf ;  e = exp(arg)
            arg = sb.tile([P, NCH * K], f32, tag="arg")
            argbc_rep = argbc_ps.unsqueeze(1).broadcast_to([P, NCH, K])
            nc.vector.tensor_tensor(arg.rearrange("p (c k) -> p c k", c=NCH),
                                    argbc_rep,
                                    lfbig.rearrange("p (c k) -> p c k", c=NCH),
                                    op=A.subtract)
            e = sb.tile([P, NCH * K], f32, tag="e")
            nc.scalar.activation(e[:], arg[:], ACT.Exp)

            # cfin[t, c] = pcol[t,c] * sum_k e[t,(c,k)]
            cfin8 = sb.tile([P, NCH], f32, tag="cfin8")
            nc.vector.tensor_reduce(
                cfin8[:], e.rearrange("p (c k) -> p c k", c=NCH),
                axis=mybir.AxisListType.X, op=A.add)
            cfin = sb.tile([P, NCH], f32, tag="cfin")
            nc.vector.tensor_tensor(cfin[:], cfin8[:], pcol[:], op=A.mult)
            cfinb = sb.tile([P, NCH], bf16, tag="cfinb")
            nc.vector.tensor_copy(cfinb[:], cfin[:])

            # out[d] = sum_c sum_t cfin[t,c] x[t,(c,d)] : 8 accumulating MMs
            out_ps = ps.tile([1, D], f32, tag="out_ps")
            for c in range(NCH):
                nc.tensor.matmul(out_ps[:], cfinb[:, c:c + 1],
                                 xa[:, c * D:(c + 1) * D],
                                 start=(c == 0), stop=(c == NCH - 1))
            out_sb = sb.tile([1, D], f32, tag="outsb")
            nc.scalar.activation(out_sb[:], out_ps[:], ACT.Copy)
            nc.sync.dma_start(out_d[:], out_sb[:])

    nc.compile()
    return nc


def _make_in_map(p, x):
    import ml_dtypes

    p = np.ascontiguousarray(np.asarray(p, dtype=np.float32)).reshape(T)
    x = np.ascontiguousarray(np.asarray(x, dtype=np.float32)).reshape(T, D)
    u, w = _gl_nodes_weights()
    aux1 = np.concatenate([(u - 1.0), np.log(w)]).astype(np.float32)
    aux1 = np.ascontiguousarray(aux1.reshape(1, 2 * K))
    p8 = np.zeros((NCH, P + NCH), np.float32)
    p8[:, 0:P] = p.reshape(NCH, P)
    p8[:, P:P + NCH] = np.eye(NCH, dtype=np.float32)
    xa = np.ascontiguousarray(
        x.reshape(NCH, P, D).transpose(1, 0, 2).reshape(P, NCH * D)
    ).astype(ml_dtypes.bfloat16)
    return {"aux1": aux1, "p8": p8, "xa": xa}


def _run(p, x, trace=False, tmpdir=None):
    from concourse.bass_utils import run_bass_kernel_spmd

    if "nc" not in _CACHE:
        _CACHE["nc"] = _build_program()
    nc = _CACHE["nc"]
    in_map = _make_in_map(p, x)
    in_maps = [in_map for _ in range(N_CORES)]
    res = run_bass_kernel_spmd(nc, in_maps, list(range(N_CORES)),
                               trace=trace, tmpdir=tmpdir)
    out = np.asarray(res.results[0]["out"], dtype=np.float32).reshape(D)
    return out, res


def kernel(p, x):
    out, _ = _run(p, x, trace=False)
    return out


# revision 4
# speedup vs baseline: 1.3781x; 1.0834x over previous
r"""Trainium2 Bass kernel for the triangular-DP "MAA layer" problem.

Reference computes, per frame t (T=1024, D=256, L=T+1 counts):
    q_t = (1-p_t) q_{t-1} + p_t shift(q_{t-1})          (Poisson-binomial DP)
    m_t = p_t a m_sh + (1-p_t) m + p_t b q_sh x_t       ([L, D] state)
    out = sum_i m_T[i, :]                               ([D])

Algebraic restructuring: the whole scan collapses to

    out[d] = sum_t c_t x[t, d],
    c_t    = p_t * I_t,   I_t = int_0^1 prod_{s != t} ((1-p_s) + p_s u) du.

The integrand is a boundary-layer spike at u=1 of width ~1/S, S = sum_s p_s.
Gauss-Legendre on the rescaled interval [1 - 30/S, 1] (host-computed from p)
converges at K=16 nodes to ~1e-12 (tail cut error e^-30).  With
f[t,k] = 1 + p_t (u_k - 1):

    slog_k = sum_t ln f[t,k]
    c_t    = p_t * sum_k exp(slog_k + ln w_k - ln f[t,k])
    out    = c^T @ x

Device mapping (t on partitions, 8 chunks of 128; k on free dim, K=16),
replicated on all 8 cores (collective latency floor exceeds compute):
  - p + identity + quadrature constants in ONE [8,168] input (8 DMA packets)
  - pcol via PE transpose-by-identity; row broadcasts via 1-partition MMs
  - single big Ln and single big Exp on ScalarE; the Exp table load hides
    behind the DVE halving-adds + PE slog/argbc matmuls
  - final contraction: 8 accumulating bf16 PE matmuls (x shipped as bf16)
  - junk warmup matmuls keep the PE busy so it ramps to the 2.4GHz p-state
    (PE runs at 0.65/1.2GHz until ~3us of continuous busy)
"""

import numpy as np

T, D, NCH, P, K = 1024, 256, 8, 128, 16
N_CORES = 8

_CACHE = {}


def _build_program():
    import concourse.bass as bass
    import concourse.bacc as bacc
    import concourse.mybir as mybir
    import concourse.tile as tile

    f32 = mybir.dt.float32
    bf16 = mybir.dt.bfloat16
    A = mybir.AluOpType
    ACT = mybir.ActivationFunctionType

    nc = bacc.Bacc("TRN2", target_bir_lowering=False, debug=False,
                   num_devices=N_CORES)

    AUXW = P + NCH + 2 * K  # [p(128) | identity(8) | um1(16) | lnw(16)]
    paux_d = nc.dram_tensor("paux", [NCH, AUXW], f32, kind="ExternalInput")
    xa_d = nc.dram_tensor("xa", [P, NCH * D], bf16, kind="ExternalInput")
    out_d = nc.dram_tensor("out", [1, D], f32, kind="ExternalOutput")

    with tile.TileContext(nc) as tc:
        with (
            tc.tile_pool(name="sb", bufs=1) as sb,
            tc.tile_pool(name="ps", bufs=1, space=bass.MemorySpace.PSUM) as ps,
        ):
            paux = sb.tile([NCH, AUXW], f32, tag="paux")
            xa = sb.tile([P, NCH * D], bf16, tag="xa")
            nc.sync.dma_start(paux[:], paux_d[:])
            nc.sync.dma_start(xa[:], xa_d[:])
            um1row = paux[0:1, P + NCH:P + NCH + K]
            lnwrow = paux[0:1, P + NCH + K:P + NCH + 2 * K]

            onesrow = sb.tile([1, P], f32, tag="onesrow")
            onescol = sb.tile([P, 1], f32, tag="onescol")
            nc.gpsimd.memset(onesrow[:], 1.0)
            nc.gpsimd.memset(onescol[:], 1.0)

            # PE warmup: no-dep junk matmuls ramp the PE p-state while the
            # input DMA is in flight.
            jmv = sb.tile([P, 512], bf16, tag="jmv")
            nc.vector.memset(jmv[:], 0.0)
            jps = ps.tile([1, 512], f32, tag="jps")
            for _ in range(3):
                nc.tensor.matmul(jps[:], jmv[:, 0:1], jmv[:],
                                 start=True, stop=True)
            nc.tensor.matmul(jps[:, 0:256], jmv[:, 0:1], jmv[:, 0:256],
                             start=True, stop=True)

            # pcol[t, c] = p[c*128 + t] via PE transpose (paux @ identity)
            pcol_ps = ps.tile([P, NCH], f32, tag="pcol_ps")
            nc.tensor.matmul(pcol_ps[:], paux[:, 0:P], paux[:, P:P + NCH],
                             start=True, stop=True)
            pcol = sb.tile([P, NCH], f32, tag="pcol")
            nc.vector.tensor_copy(pcol[:], pcol_ps[:])

            # um1bc[t, k] = u_k - 1 broadcast across partitions
            um1bc_ps = ps.tile([P, K], f32, tag="um1bc_ps")
            nc.tensor.matmul(um1bc_ps[:], onesrow[:], um1row,
                             start=True, stop=True)

            # fm1[t,(c,k)] = pcol[t,c] * um1[k]
            fm1 = sb.tile([P, NCH * K], f32, tag="fm1")
            um1_rep = um1bc_ps.unsqueeze(1).broadcast_to([P, NCH, K])
            p_rep = pcol.unsqueeze(2).broadcast_to([P, NCH, K])
            nc.vector.tensor_tensor(fm1.rearrange("p (c k) -> p c k", c=NCH),
                                    um1_rep, p_rep, op=A.mult)

            # lf = Ln(fm1 + 1)
            lfbig = sb.tile([P, NCH * K], f32, tag="lfbig")
            nc.scalar.activation(lfbig[:], fm1[:], ACT.Ln, bias=1.0)

            # lfsum[t, k] = sum_c lf[t,(c,k)] via 3 contiguous halving adds
            h1 = sb.tile([P, 4 * K], f32, tag="h1")
            nc.vector.tensor_tensor(h1[:], lfbig[:, 0:4 * K],
                                    lfbig[:, 4 * K:8 * K], op=A.add)
            h2 = sb.tile([P, 2 * K], f32, tag="h2")
            nc.vector.tensor_tensor(h2[:], h1[:, 0:2 * K], h1[:, 2 * K:4 * K],
                                    op=A.add)
            lfsum = sb.tile([P, K], f32, tag="lfsum")
            nc.vector.tensor_tensor(lfsum[:], h2[:, 0:K], h2[:, K:2 * K],
                                    op=A.add)

            # slog_k = sum_t lfsum[t, k]  (PE partition reduce)
            slog_ps = ps.tile([1, K], f32, tag="slog_ps")
            nc.tensor.matmul(slog_ps[:], onescol[:], lfsum[:],
                             start=True, stop=True)

            # slnw = slog + ln w ; broadcast to partitions on PE
            slnw = sb.tile([1, K], f32, tag="slnw")
            nc.vector.tensor_tensor(slnw[:], slog_ps[:], lnwrow, op=A.add)
            argbc_ps = ps.tile([P, K], f32, tag="argbc_ps")
            nc.tensor.matmul(argbc_ps[:], onesrow[:], slnw[:],
                             start=True, stop=True)

            # PE gap filler between argbc and the final matmuls (keeps the
            # high p-state; moving operand dep on h1 keeps it mid-schedule)
            jps2 = ps.tile([1, 4 * K], f32, tag="jps2")
            for _ in range(5):
                nc.tensor.matmul(jps2[:], onescol[:], h1[:, 0:4 * K],
                                 start=True, stop=True)

            # arg = (slog + lnw) - lf ;  e = exp(arg)
            arg = sb.tile([P, NCH * K], f32, tag="arg")
            argbc_rep = argbc_ps.unsqueeze(1).broadcast_to([P, NCH, K])
            nc.vector.tensor_tensor(arg.rearrange("p (c k) -> p c k", c=NCH),
                                    argbc_rep,
                                    lfbig.rearrange("p (c k) -> p c k", c=NCH),
                                    op=A.subtract)
            e = sb.tile([P, NCH * K], f32, tag="e")
            nc.scalar.activation(e[:], arg[:], ACT.Exp)

            # cfin[t, c] = pcol[t,c] * sum_k e[t,(c,k)]  (bf16 out, fused)
            cfin8 = sb.tile([P, NCH], f32, tag="cfin8")
            nc.vector.tensor_reduce(
                cfin8[:], e.rearrange("p (c k) -> p c k", c=NCH),
                axis=mybir.AxisListType.X, op=A.add)
            cfinb = sb.tile([P, NCH], bf16, tag="cfinb")
            nc.vector.tensor_tensor(cfinb[:], cfin8[:], pcol[:], op=A.mult)

            # out[d] = sum_c sum_t cfin[t,c] x[t,(c,d)] : 8 accumulating MMs
            out_ps = ps.tile([1, D], f32, tag="out_ps")
            for c in range(NCH):
                nc.tensor.matmul(out_ps[:], cfinb[:, c:c + 1],
                                 xa[:, c * D:(c + 1) * D],
                                 start=(c == 0), stop=(c == NCH - 1))
            out_sb = sb.tile([1, D], f32, tag="outsb")
            nc.scalar.activation(out_sb[:], out_ps[:], ACT.Copy)
            nc.sync.dma_start(out_d[:], out_sb[:])

    nc.compile()
    return nc


def _make_in_map(p, x):
    import ml_dtypes

    p = np.ascontiguousarray(np.asarray(p, dtype=np.float32)).reshape(T)
    x = np.ascontiguousarray(np.asarray(x, dtype=np.float32)).reshape(T, D)
    S = float(np.float64(p).sum()) if p.ndim else float(p.sum())
    S = float(np.sum(np.asarray(p, np.float64)))
    delta = min(1.0, 30.0 / max(S, 1.0))
    nodes, weights = np.polynomial.legendre.leggauss(K)
    u = 1.0 - delta + delta * (nodes + 1.0) * 0.5
    w = weights * delta * 0.5
    paux = np.zeros((NCH, P + NCH + 2 * K), np.float32)
    paux[:, 0:P] = p.reshape(NCH, P)
    paux[:, P:P + NCH] = np.eye(NCH, dtype=np.float32)
    paux[0, P + NCH:P + NCH + K] = (u - 1.0).astype(np.float32)
    paux[0, P + NCH + K:P + NCH + 2 * K] = np.log(w).astype(np.float32)
    xa = np.ascontiguousarray(
        x.reshape(NCH, P, D).transpose(1, 0, 2).reshape(P, NCH * D)
    ).astype(ml_dtypes.bfloat16)
    return {"paux": paux, "xa": xa}


def _run(p, x, trace=False, tmpdir=None):
    from concourse.bass_utils import run_bass_kernel_spmd

    if "nc" not in _CACHE:
        _CACHE["nc"] = _build_program()
    nc = _CACHE["nc"]
    in_map = _make_in_map(p, x)
    in_maps = [in_map for _ in range(N_CORES)]
    res = run_bass_kernel_spmd(nc, in_maps, list(range(N_CORES)),
                               trace=trace, tmpdir=tmpdir)
    out = np.asarray(res.results[0]["out"], dtype=np.float32).reshape(D)
    return out, res


def kernel(p, x):
    out, _ = _run(p, x, trace=False)
    return out
